# revision 1
# baseline (speedup 1.0000x reference)
"""BinaryLinear on 8 TRN2 NeuronCores.

y = mean(|W|) * (x @ sign(W)^T),  x:[8192,4096] f32, W:[4096,4096] f32.

Strategy (batch-parallel): shard x rows across 8 cores (1024 each); every core
reads the full W, binarizes + transposes it on the fly (PE fp32 transpose ->
ACT Sign -> bf16 +-1/0 staged stripe), and runs bf16 matmuls against a
resident transposed-x (built once via PE transpose). sign(W) in {-1,0,+1} is
exact in bf16, so products are exact and only the bf16 rounding of x
contributes error. The scale mean(|W|) is computed from a per-core W row
slice + AllReduce, and applied on the PSUM->SBUF copy of the output.
"""
from contextlib import ExitStack

import numpy as np

import concourse.bass as bass
import concourse.mybir as mybir
import concourse.tile as tile
from concourse import bacc
from concourse.bass_utils import run_bass_kernel_spmd
from concourse.masks import make_identity

P = 128
B, IN, OUT = 8192, 4096, 4096
NCORES = 8
BSH = B // NCORES          # 1024 batch rows per core
M_TILES = BSH // P         # 8
K_TILES = IN // P          # 32
N_TILE = 512
N_TILES = OUT // N_TILE    # 8
KO_GROUPS = 8              # k handled in groups of 4 k-tiles (512 cols)
WSLICE = OUT // NCORES     # 512 W rows per core for the scale prepass

F32 = mybir.dt.float32
BF16 = mybir.dt.bfloat16

_cache = {}


def _build(repeat=1, st_mode="pe2", use_collective=True, phases="AB",
           wload_bufs=4, acc_bufs=4, stg_bufs=2, pst_bufs=2,
           lookahead=False, tcast=False):
    """st_mode: how S^T stripes are produced.
    "pe"  — PE transpose of fp32 W tiles + ACT Sign on the PSUM copyback.
    "dma" — ACT Sign to a natural-layout bf16 DRAM bounce + DMA-transpose reads.
    use_collective=False replaces the AllReduce with a local copy (wrong scale
    by 8x, only for single-core TimelineSim).
    """
    nc = bacc.Bacc("TRN2", target_bir_lowering=False, debug=False, num_devices=NCORES)

    x_ext = nc.dram_tensor("x", [BSH, IN], F32, kind="ExternalInput").ap()
    w_ext = nc.dram_tensor("w", [OUT, IN], F32, kind="ExternalInput").ap()
    wsl_ext = nc.dram_tensor("wsl", [WSLICE, IN], F32, kind="ExternalInput").ap()
    y_ext = nc.dram_tensor("y", [BSH, OUT], F32, kind="ExternalOutput").ap()

    with tile.TileContext(nc) as tc, ExitStack() as ctx:
        consts = ctx.enter_context(tc.tile_pool(name="consts", bufs=1))
        xT_pool = ctx.enter_context(tc.tile_pool(name="xT", bufs=1))
        xload = ctx.enter_context(tc.tile_pool(name="xload", bufs=2))
        wload = ctx.enter_context(tc.tile_pool(name="wload", bufs=wload_bufs))
        stg_pool = ctx.enter_context(tc.tile_pool(name="stg", bufs=stg_bufs))
        y_pool = ctx.enter_context(tc.tile_pool(name="y", bufs=3))
        psT = ctx.enter_context(tc.tile_pool(name="psT", bufs=pst_bufs, space="PSUM"))
        accp = ctx.enter_context(tc.tile_pool(name="acc", bufs=acc_bufs, space="PSUM"))
        dram = ctx.enter_context(tc.tile_pool(name="dram", bufs=2, space="DRAM"))

        identity = consts.tile([P, P], F32)
        make_identity(nc, identity)
        identity_bf = consts.tile([P, P], BF16)
        make_identity(nc, identity_bf)
        ones = consts.tile([P, P], F32)
        nc.gpsimd.memset(ones, 1.0)

        # ---- scale prepass: sum |wsl| on this core, AllReduce across cores ----
        wsl_v = wsl_ext.rearrange("(c p) k -> p c k", p=P)  # [128, 4, 4096]
        CH = 2048
        NCH = (WSLICE // P) * IN // CH  # 8 chunks
        partials = consts.tile([P, NCH], F32)
        for i in range(NCH):
            c, h = divmod(i, IN // CH)
            wc = wload.tile([P, CH], F32, tag="wl")
            nc.sync.dma_start(wc[:], wsl_v[:, c, h * CH:(h + 1) * CH])
            trash = wload.tile([P, CH], F32, tag="wl")
            nc.scalar.activation(trash[:], wc[:], mybir.ActivationFunctionType.Abs,
                                 accum_out=partials[:, i:i + 1])
        # reduce the NCH partials (values >= 0, Abs is a no-op here)
        partial1 = consts.tile([P, 1], F32)
        trash2 = consts.tile([P, NCH], F32)
        nc.scalar.activation(trash2[:], partials[:], mybir.ActivationFunctionType.Abs,
                             accum_out=partial1[:])
        ar_in = dram.tile([P, 1], F32)
        ar_res = dram.tile([P, 1], F32)
        nc.sync.dma_start(ar_in[:], partial1[:])
        if use_collective:
            nc.gpsimd.collective_compute(
                "AllReduce", mybir.AluOpType.add,
                replica_groups=[list(range(NCORES))],
                ins=[ar_in.opt()], outs=[ar_res.opt()],
            )
        else:
            nc.sync.dma_start(ar_res[:], ar_in[:])
        ar_sb = consts.tile([P, 1], F32)
        nc.sync.dma_start(ar_sb[:], ar_res[:])
        # broadcast sum across partitions: ones.T @ ar_sb, then * 1/(OUT*IN)
        ps_bc = accp.tile([P, N_TILE], F32, tag="acc")
        nc.tensor.matmul(ps_bc[:, 0:1], ones[:], ar_sb[:], start=True, stop=True)
        scale_sb = consts.tile([P, 1], F32)
        nc.scalar.mul(scale_sb[:], ps_bc[:, 0:1], 1.0 / float(OUT * IN))

        # ---- xT: resident transposed x, bf16 [128(k), K_TILES, BSH] ----
        PST_DT = BF16 if tcast else F32
        IDENT = identity_bf if tcast else identity
        xT = xT_pool.tile([P, K_TILES, BSH], BF16)
        XCH = 2048  # x row-stripe loaded in 2 chunks of 16 k-tiles
        for m in range(M_TILES):
            for h in range(IN // XCH):
                xl = xload.tile([P, XCH], F32)
                nc.sync.dma_start(
                    xl[:], x_ext[m * P:(m + 1) * P, h * XCH:(h + 1) * XCH])
                if tcast:
                    xsrc = xload.tile([P, XCH], BF16, tag="xlb")
                    nc.vector.tensor_copy(out=xsrc[:], in_=xl[:])
                else:
                    xsrc = xl
                for kq in range(XCH // P // 4):
                    k0 = h * (XCH // P) + kq * 4
                    pst = psT.tile([P, N_TILE], PST_DT, tag="pst")
                    for j in range(4):
                        nc.tensor.transpose(
                            pst[:, j * P:(j + 1) * P],
                            xsrc[:, (kq * 4 + j) * P:(kq * 4 + j + 1) * P],
                            IDENT[:])
                    nc.vector.tensor_copy(
                        out=xT[:, k0:k0 + 4, m * P:(m + 1) * P],
                        in_=pst.rearrange("p (j f) -> p j f", j=4)[:])

        # ---- main loop over output column tiles ----
        # W viewed as [n_tile, ko, p(out), c(out), f(k)]
        w_v = w_ext.rearrange("(n c p) (ko f) -> n ko p c f", p=P, c=N_TILE // P,
                              f=IN // KO_GROUPS)
        def emit_A(nj):
            stg = stg_pool.tile([P, K_TILES, N_TILE], BF16)
            if "A" not in phases:
                nc.gpsimd.memset(stg[:], 1.0)
                return stg
            if st_mode == "pe":
                for ko in range(KO_GROUPS):
                    wl = wload.tile([P, N_TILE // P, IN // KO_GROUPS], F32, tag="wl")
                    nc.sync.dma_start(wl[:], w_v[nj, ko])
                    for c in range(N_TILE // P):
                        for kk in range(IN // KO_GROUPS // P):
                            k = ko * (IN // KO_GROUPS // P) + kk
                            pst = psT.tile([P, N_TILE], F32, tag="pst")
                            nc.tensor.transpose(
                                pst[:, 0:P], wl[:, c, kk * P:(kk + 1) * P], identity[:])
                            nc.scalar.sign(stg[:, k, c * P:(c + 1) * P], pst[:, 0:P])
            elif st_mode == "pe2":
                for ko in range(KO_GROUPS):
                    wl = wload.tile([P, N_TILE // P, IN // KO_GROUPS], F32, tag="wl")
                    nc.sync.dma_start(wl[:, 0:2], w_v[nj, ko][:, 0:2])
                    nc.sync.dma_start(wl[:, 2:4], w_v[nj, ko][:, 2:4])
                    if tcast:
                        src_t = wload.tile(
                            [P, N_TILE // P, IN // KO_GROUPS], BF16, tag="wlb")
                        nc.vector.tensor_copy(out=src_t[:], in_=wl[:])
                    else:
                        src_t = wl
                    for kk in range(IN // KO_GROUPS // P):
                        k = ko * (IN // KO_GROUPS // P) + kk
                        pst = psT.tile([P, N_TILE], PST_DT, tag="pst")
                        for c in range(N_TILE // P):
                            nc.tensor.transpose(
                                pst[:, c * P:(c + 1) * P],
                                src_t[:, c, kk * P:(kk + 1) * P], IDENT[:])
                        nc.scalar.sign(stg[:, k, :], pst[:])
            else:
                sdram = dram.tile([N_TILE, IN], BF16, tag="sdram")
                sdram_v = sdram.rearrange("(c p) k -> p c k", p=P)
                for ko in range(KO_GROUPS):
                    wl = wload.tile([P, N_TILE // P, IN // KO_GROUPS], F32, tag="wl")
                    nc.sync.dma_start(wl[:], w_v[nj, ko])
                    sg = wload.tile([P, N_TILE // P, IN // KO_GROUPS], BF16, tag="sg")
                    nc.scalar.sign(sg[:], wl[:])
                    nc.sync.dma_start(
                        sdram_v[:, :, ko * (IN // KO_GROUPS):(ko + 1) * (IN // KO_GROUPS)],
                        sg[:])
                for k in range(K_TILES):
                    nc.scalar.dma_start_transpose(
                        stg[:, k, :], sdram[:, k * P:(k + 1) * P])
            return stg

        def emit_B(nj, stg):
            if "B" not in phases:
                return
            for m in range(M_TILES):
                acc = accp.tile([P, N_TILE], F32, tag="acc")
                for k in range(K_TILES):
                    nc.tensor.matmul(
                        acc[:], xT[:, k, m * P:(m + 1) * P], stg[:, k, :],
                        start=(k == 0), stop=(k == K_TILES - 1))
                yt = y_pool.tile([P, N_TILE], F32)
                nc.vector.tensor_scalar_mul(yt[:], acc[:], scale_sb[:])
                nc.sync.dma_start(
                    y_ext[m * P:(m + 1) * P, nj * N_TILE:(nj + 1) * N_TILE], yt[:])

        nj_list = [nj for _ in range(repeat) for nj in range(N_TILES)]
        if lookahead:
            stg_cur = emit_A(nj_list[0])
            for i, nj in enumerate(nj_list):
                stg_next = emit_A(nj_list[i + 1]) if i + 1 < len(nj_list) else None
                emit_B(nj, stg_cur)
                stg_cur = stg_next
        else:
            for nj in nj_list:
                emit_B(nj, emit_A(nj))

    nc.finalize()
    return nc


def kernel(x: np.ndarray, weight: np.ndarray) -> np.ndarray:
    if "nc" not in _cache:
        _cache["nc"] = _build(lookahead=True, acc_bufs=6)
    nc = _cache["nc"]

    x = np.ascontiguousarray(x, dtype=np.float32)
    weight = np.ascontiguousarray(weight, dtype=np.float32)
    in_maps = []
    for c in range(NCORES):
        in_maps.append({
            "x": x[c * BSH:(c + 1) * BSH],
            "w": weight,
            "wsl": weight[c * WSLICE:(c + 1) * WSLICE],
        })
    res = run_bass_kernel_spmd(nc, in_maps, list(range(NCORES)))
    _cache["last_results"] = res
    return np.concatenate([res.results[c]["y"] for c in range(NCORES)], axis=0)

